# revision 28
# baseline (speedup 1.0000x reference)
"""LIF spiking-neuron recurrence kernel for Trainium2 (8 NeuronCores, SPMD).

Problem: x [32, 100, 8192] f32, decay [1] f32.
    d = sigmoid(decay)
    mem_0 = x[:,0];  mem_t = mem_{t-1} * d * (1 - spike_{t-1}) + x[:,t]
    spike_t = (mem_t > 0.5);  out[:,t] = spike_t  (f32 0/1)

Device formulation (bit-exact vs the reference):
    W_{-1} = 0
    M_t = (W_{t-1} * d) + x_t
    W_t = (M_t <= 0.5) * M_t
spike_t = (M_t > 0.5) = (W_t == 0) exactly (W_t = M_t != 0 when no spike,
= +0.0 when spike). The device computes spike = (W == 0) with a cheap
tensor_scalar pass that writes uint8 directly (DVE cast-on-write) and
streams it out over the plain HWDGE path — a 4x smaller output than f32
without the slow SWDGE/CCE cast-in-DMA.

The whole step is ONE custom DVE op (registered at runtime through the
concourse custom-DVE table mechanism):
    LIF_STEP_ANT: out = M * (M <= s1),  M = in0*s0 + in1
Each ALU stage rounds in f32 exactly like the reference's mult/add chain,
and the *(0/1) mask multiply is exact, so results match the reference
bit-for-bit.

Sharding: the 32*8192 = 262144 independent (b, d) lanes are split 8 ways by
feature blocks (d-shard): core c owns d in [1024c, 1024c+1024). Per-core
layout is [128 partitions, T*256] with partition p = b*4 + (d_local//256),
free offset = t*256 + d_local%256, so each timestep is a [128, 256] slice
and DMA lines are long and contiguous. No cross-core communication
(forward only).

Chunked DMA schedule: a small first chunk so compute starts early, bulk
~21-step chunks (2.7 MB loads), small last chunk so the tail flush is
short. Input loads issue from the Scalar-engine HWDGE ring, output stores
from the Sync-engine HWDGE ring; both avoid the slow SWDGE cast path.
Each chunk runs as ONE self-referential LIF instruction over the
persistent W buffer plus one spike-conversion pass.

Measured on the 8-core axon TRN2 pod: ~61-66 us NEFF exec time,
bit-exact vs the jax reference (0 / 26.2M element mismatches).
"""

from contextlib import ExitStack

import numpy as np

N_CORES = 8
B, T, D = 32, 100, 8192
P = 128          # SBUF partitions
F = 256          # free elements per timestep per core (32*1024/128)
THRESH = 0.5
DVE_SPIKE_FRAC = 1.0    # fraction of spike-conversion columns done on DVE
                        # (rest goes to the ACT engine; 1.0 = all on DVE)

_BUILD_CACHE: dict = {}
_LIF_OP = None


def _chunk_schedule(t_steps: int) -> list[int]:
    if t_steps == 100:
        return [4, 12, 21, 21, 21, 17, 4]
    chunks = []
    rem = t_steps
    while rem > 0:
        c = min(20, rem)
        chunks.append(c)
        rem -= c
    return chunks


def _get_lif_op():
    """Register the fused LIF-step custom DVE op (idempotent)."""
    global _LIF_OP
    if _LIF_OP is not None:
        return _LIF_OP
    from concourse.dve_ops import (
        CUSTOM_DVE_SPECS, OPS, _SUB_OPCODE_FOR_NAME, DveOp,
    )
    from concourse.dve_spec import C0, C1, Spec, Src0, Src1, lower
    from concourse.dve_table_gen import dve_ver_for
    from concourse.dve_uop import DveOpSpec

    name = "LIF_STEP_ANT"
    if name in _SUB_OPCODE_FOR_NAME:
        _LIF_OP = next(op for op in OPS if op.name == name)
        return _LIF_OP

    M = Src0 * C0 + Src1

    def _ref(in0, in1, s0, s1, imm2):
        m = (in0.astype(np.float32) * np.float32(s0)
             + in1.astype(np.float32)).astype(np.float32)
        return np.where(m <= np.float32(s1), m, np.float32(0.0)).astype(np.float32)

    spec = Spec(body=M * (M <= C1), reference=_ref)
    row = max(_SUB_OPCODE_FOR_NAME.values()) + 1
    assert row < 0x20
    _SUB_OPCODE_FOR_NAME[name] = row
    shas = {}
    for ver in ("v3",):  # TRN2
        tmp = DveOpSpec(name=name, opcode=row, uops=lower(spec, ver=ver),
                        rd1_en=True)
        shas[ver] = tmp.sha(ver)
    assert dve_ver_for("TRN2") == "v3"
    op = DveOp(name, spec, subdim=False, uops_sha=shas)
    OPS.append(op)
    CUSTOM_DVE_SPECS[name] = spec
    _LIF_OP = op
    return op


def _build_nc(t_steps: int, d_imm: float):
    import concourse.tile as tile
    from concourse import bacc, mybir

    lif_op = _get_lif_op()
    chunks = _chunk_schedule(t_steps)
    assert sum(chunks) == t_steps
    max_tc = max(chunks)

    nc = bacc.Bacc("TRN2", debug=False, target_bir_lowering=False)
    x_in = nc.dram_tensor("x", [P, t_steps * F], mybir.dt.float32,
                          kind="ExternalInput")
    s_out = nc.dram_tensor("s", [P, t_steps * F], mybir.dt.uint8,
                           kind="ExternalOutput")

    act_assist = DVE_SPIKE_FRAC < 1.0
    qsize = max(F, (int(max_tc * F * (1 - DVE_SPIKE_FRAC)) // F + 1) * F)

    with tile.TileContext(nc) as tcx, ExitStack() as ctx:
        xpool = ctx.enter_context(
            tcx.tile_pool(name="xp", bufs=2 if act_assist else 3))
        opool = ctx.enter_context(tcx.tile_pool(name="op", bufs=3))
        qpool = (ctx.enter_context(tcx.tile_pool(name="qp", bufs=2))
                 if act_assist else None)
        spool = ctx.enter_context(tcx.tile_pool(name="sp", bufs=1))

        # Persistent state buffer: W[:, t*F:(t+1)*F] holds W_{t-1} (so slot 0
        # is the zero initial state and slot t+1 is W_t).
        wbuf = spool.tile([P, (t_steps + 1) * F], mybir.dt.float32)
        nc.vector.memset(wbuf[:, 0:F], 0.0)

        t0 = 0
        for tc in chunks:
            xt = xpool.tile([P, max_tc * F], mybir.dt.float32, tag="xt")
            nc.scalar.dma_start(out=xt[:, :tc * F],
                                in_=x_in[:, t0 * F:(t0 + tc) * F])
            # One DVE instruction runs tc recurrence steps: the out AP trails
            # the in0 AP by exactly F elements in the same buffer, so the
            # write of W_t lands ~250 cycles before W_t is read back for
            # step t+1 (verified bit-exact on HW).
            nc.vector._custom_dve(
                lif_op,
                out=wbuf[:, (t0 + 1) * F:(t0 + tc + 1) * F],
                in0=wbuf[:, t0 * F:(t0 + tc) * F],
                in1=xt[:, :tc * F],
                s0=d_imm, s1=THRESH)
            # spike = (W == 0) as uint8, split between DVE (tensor_scalar
            # is_equal at 2x) and the otherwise-idle ACT engine. ACT has no
            # compare, but q = Square(1e19*W) maps W==0 -> 0 and any real
            # nonzero W (|W| >= 1e-19; actual values are ~1e-3..10) to >= 1
            # (or inf), so spike = Relu(1 - q) is exactly 1.0/0.0.
            n = tc * F
            c1 = (int(n * DVE_SPIKE_FRAC) // F) * F
            wslice = wbuf[:, (t0 + 1) * F:(t0 + tc + 1) * F]
            st = opool.tile([P, max_tc * F], mybir.dt.uint8, tag="st")
            if c1 > 0:
                nc.vector.tensor_scalar(
                    out=st[:, :c1], in0=wslice[:, :c1],
                    scalar1=0.0, scalar2=None, op0=mybir.AluOpType.is_equal)
            if c1 < n:
                qt = qpool.tile([P, qsize], mybir.dt.float32, tag="qt")
                nc.scalar.activation(
                    out=qt[:, :n - c1], in_=wslice[:, c1:n],
                    func=mybir.ActivationFunctionType.Square, scale=1e19)
                nc.scalar.activation(
                    out=st[:, c1:n], in_=qt[:, :n - c1],
                    func=mybir.ActivationFunctionType.Relu,
                    bias=1.0, scale=-1.0)
            nc.sync.dma_start(out=s_out[:, t0 * F:(t0 + tc) * F],
                              in_=st[:, :tc * F])
            t0 += tc
    nc.compile()
    return nc


def _get_nc(t_steps: int, d_imm: float):
    key = (t_steps, np.float32(d_imm).tobytes())
    if key not in _BUILD_CACHE:
        _BUILD_CACHE[key] = _build_nc(t_steps, d_imm)
    return _BUILD_CACHE[key]


def _shard_x(x: np.ndarray) -> list[np.ndarray]:
    b, t, d = x.shape
    # [b, t, core, chunk, 256] -> [core, b, chunk, t, 256] -> [core, 128, t*256]
    xr = x.reshape(b, t, N_CORES, 4, F).transpose(2, 0, 3, 1, 4)
    xr = np.ascontiguousarray(xr).reshape(N_CORES, P, t * F)
    return [xr[c] for c in range(N_CORES)]


def _unshard_spikes(s8: np.ndarray, t: int) -> np.ndarray:
    # s8 already holds spikes as u8 0/1; [core, 128, t*256] -> [b, t, D]
    sr = s8.astype(np.float32).reshape(N_CORES, B, 4, t, F)
    sr = sr.transpose(1, 3, 0, 2, 4)
    return np.ascontiguousarray(sr).reshape(B, t, N_CORES * 4 * F)


def _sigmoid_f32(decay: np.ndarray) -> np.float32:
    import jax
    import jax.numpy as jnp
    d = np.asarray(jax.nn.sigmoid(jnp.asarray(decay, jnp.float32)))
    return np.float32(d.reshape(-1)[0])


def kernel(x: np.ndarray, decay: np.ndarray) -> np.ndarray:
    from concourse.bass_utils import run_bass_kernel_spmd

    x = np.asarray(x, dtype=np.float32)
    b, t, d = x.shape
    d_f32 = _sigmoid_f32(np.asarray(decay))

    nc = _get_nc(t, float(d_f32))
    shards = _shard_x(x)
    in_maps = [{"x": np.ascontiguousarray(s)} for s in shards]
    res = run_bass_kernel_spmd(nc, in_maps, core_ids=list(range(N_CORES)))
    s8 = np.stack([np.asarray(res.results[c]["s"]) for c in range(N_CORES)],
                  axis=0)
    return _unshard_spikes(s8, t)


# revision 29
# speedup vs baseline: 1.1193x; 1.1193x over previous
"""LIF spiking-neuron recurrence kernel for Trainium2 (8 NeuronCores, SPMD).

Problem: x [32, 100, 8192] f32, decay [1] f32.
    d = sigmoid(decay)
    mem_0 = x[:,0];  mem_t = mem_{t-1} * d * (1 - spike_{t-1}) + x[:,t]
    spike_t = (mem_t > 0.5);  out[:,t] = spike_t  (f32 0/1)

Device formulation (bit-exact vs the reference):
    W_{-1} = 0
    M_t = (W_{t-1} * d) + x_t
    W_t = (M_t <= 0.5) * M_t
spike_t = (M_t > 0.5) = (W_t == 0) exactly (W_t = M_t != 0 when no spike,
= +0.0 when spike). The device computes spike = (W == 0) with a cheap
tensor_scalar pass that writes uint8 directly (DVE cast-on-write) and
streams it out over the plain HWDGE path — a 4x smaller output than f32
without the slow SWDGE/CCE cast-in-DMA.

The whole step is ONE custom DVE op (registered at runtime through the
concourse custom-DVE table mechanism):
    LIF_STEP_ANT: out = M * (M <= s1),  M = in0*s0 + in1
Each ALU stage rounds in f32 exactly like the reference's mult/add chain,
and the *(0/1) mask multiply is exact, so results match the reference
bit-for-bit.

Sharding: the 32*8192 = 262144 independent (b, d) lanes are split 8 ways by
feature blocks (d-shard): core c owns d in [1024c, 1024c+1024). Per-core
layout is [128 partitions, T*256] with partition p = b*4 + (d_local//256),
free offset = t*256 + d_local%256, so each timestep is a [128, 256] slice
and DMA lines are long and contiguous. No cross-core communication
(forward only).

Chunked DMA schedule: a small first chunk so compute starts early, bulk
~21-step chunks (2.7 MB loads), small last chunk so the tail flush is
short. Input loads issue from the Scalar-engine HWDGE ring, output stores
from the Sync-engine HWDGE ring; both avoid the slow SWDGE cast path.
Each chunk runs as ONE self-referential LIF instruction over the
persistent W buffer plus one spike-conversion pass.

Measured on the 8-core axon TRN2 pod: ~61-66 us NEFF exec time,
bit-exact vs the jax reference (0 / 26.2M element mismatches).
"""

from contextlib import ExitStack

import numpy as np

N_CORES = 8
B, T, D = 32, 100, 8192
P = 128          # SBUF partitions
F = 256          # free elements per timestep per core (32*1024/128)
THRESH = 0.5
DVE_SPIKE_FRAC = 1.0    # fraction of spike-conversion columns done on DVE
                        # (rest goes to the ACT engine; 1.0 = all on DVE)

_BUILD_CACHE: dict = {}
_LIF_OP = None


def _chunk_schedule(t_steps: int) -> list[int]:
    if t_steps == 100:
        return [4, 12, 21, 21, 21, 17, 4]
    chunks = []
    rem = t_steps
    while rem > 0:
        c = min(20, rem)
        chunks.append(c)
        rem -= c
    return chunks


def _get_lif_op():
    """Register the fused LIF-step custom DVE op (idempotent)."""
    global _LIF_OP
    if _LIF_OP is not None:
        return _LIF_OP
    from concourse.dve_ops import (
        CUSTOM_DVE_SPECS, OPS, _SUB_OPCODE_FOR_NAME, DveOp,
    )
    from concourse.dve_spec import C0, C1, Spec, Src0, Src1, lower
    from concourse.dve_table_gen import dve_ver_for
    from concourse.dve_uop import DveOpSpec

    name = "LIF_STEP_ANT"
    if name in _SUB_OPCODE_FOR_NAME:
        _LIF_OP = next(op for op in OPS if op.name == name)
        return _LIF_OP

    M = Src0 * C0 + Src1

    def _ref(in0, in1, s0, s1, imm2):
        m = (in0.astype(np.float32) * np.float32(s0)
             + in1.astype(np.float32)).astype(np.float32)
        return np.where(m <= np.float32(s1), m, np.float32(0.0)).astype(np.float32)

    spec = Spec(body=M * (M <= C1), reference=_ref)
    row = max(_SUB_OPCODE_FOR_NAME.values()) + 1
    assert row < 0x20
    _SUB_OPCODE_FOR_NAME[name] = row
    shas = {}
    for ver in ("v3",):  # TRN2
        tmp = DveOpSpec(name=name, opcode=row, uops=lower(spec, ver=ver),
                        rd1_en=True)
        shas[ver] = tmp.sha(ver)
    assert dve_ver_for("TRN2") == "v3"
    op = DveOp(name, spec, subdim=False, uops_sha=shas)
    OPS.append(op)
    CUSTOM_DVE_SPECS[name] = spec
    _LIF_OP = op
    return op


def _build_nc(t_steps: int, d_imm: float):
    import concourse.tile as tile
    from concourse import bacc, mybir

    lif_op = _get_lif_op()
    chunks = _chunk_schedule(t_steps)
    assert sum(chunks) == t_steps
    max_tc = max(chunks)

    nc = bacc.Bacc("TRN2", debug=False, target_bir_lowering=False)
    x_in = nc.dram_tensor("x", [P, t_steps * F], mybir.dt.float32,
                          kind="ExternalInput")
    s_out = nc.dram_tensor("s", [P, t_steps * F], mybir.dt.uint8,
                           kind="ExternalOutput")

    act_assist = DVE_SPIKE_FRAC < 1.0
    qsize = max(F, (int(max_tc * F * (1 - DVE_SPIKE_FRAC)) // F + 1) * F)

    with tile.TileContext(nc) as tcx, ExitStack() as ctx:
        xpool = ctx.enter_context(
            tcx.tile_pool(name="xp", bufs=2 if act_assist else 3))
        opool = ctx.enter_context(tcx.tile_pool(name="op", bufs=3))
        qpool = (ctx.enter_context(tcx.tile_pool(name="qp", bufs=2))
                 if act_assist else None)
        spool = ctx.enter_context(tcx.tile_pool(name="sp", bufs=1))

        # Persistent state buffer: W[:, t*F:(t+1)*F] holds W_{t-1} (so slot 0
        # is the zero initial state and slot t+1 is W_t).
        wbuf = spool.tile([P, (t_steps + 1) * F], mybir.dt.float32)
        nc.vector.memset(wbuf[:, 0:F], 0.0)

        def emit_spike_out(t0, tc):
            # spike = (W == 0) as uint8, split between DVE (tensor_scalar
            # is_equal at 2x) and the otherwise-idle ACT engine. ACT has no
            # compare, but q = Square(1e19*W) maps W==0 -> 0 and any real
            # nonzero W (|W| >= 1e-19; actual values are ~1e-3..10) to >= 1
            # (or inf), so spike = Relu(1 - q) is exactly 1.0/0.0.
            n = tc * F
            c1 = (int(n * DVE_SPIKE_FRAC) // F) * F
            wslice = wbuf[:, (t0 + 1) * F:(t0 + tc + 1) * F]
            st = opool.tile([P, max_tc * F], mybir.dt.uint8, tag="st")
            if c1 > 0:
                nc.vector.tensor_scalar(
                    out=st[:, :c1], in0=wslice[:, :c1],
                    scalar1=0.0, scalar2=None, op0=mybir.AluOpType.is_equal)
            if c1 < n:
                qt = qpool.tile([P, qsize], mybir.dt.float32, tag="qt")
                nc.scalar.activation(
                    out=qt[:, :n - c1], in_=wslice[:, c1:n],
                    func=mybir.ActivationFunctionType.Square, scale=1e19)
                nc.scalar.activation(
                    out=st[:, c1:n], in_=qt[:, :n - c1],
                    func=mybir.ActivationFunctionType.Relu,
                    bias=1.0, scale=-1.0)
            nc.sync.dma_start(out=s_out[:, t0 * F:(t0 + tc) * F],
                              in_=st[:, :tc * F])

        t0 = 0
        pending = None  # defer each chunk's spike pass until after the next
        for tc in chunks:
            xt = xpool.tile([P, max_tc * F], mybir.dt.float32, tag="xt")
            nc.scalar.dma_start(out=xt[:, :tc * F],
                                in_=x_in[:, t0 * F:(t0 + tc) * F])
            # One DVE instruction runs tc recurrence steps: the out AP trails
            # the in0 AP by exactly F elements in the same buffer, so the
            # write of W_t lands ~250 cycles before W_t is read back for
            # step t+1 (verified bit-exact on HW).
            nc.vector._custom_dve(
                lif_op,
                out=wbuf[:, (t0 + 1) * F:(t0 + tc + 1) * F],
                in0=wbuf[:, t0 * F:(t0 + tc) * F],
                in1=xt[:, :tc * F],
                s0=d_imm, s1=THRESH)
            # Emit the PREVIOUS chunk's spike conversion after this chunk's
            # LIF so the serial LIF chain gets DVE priority; spike passes
            # fill the slots where the LIF would otherwise wait on DMA.
            if pending is not None:
                emit_spike_out(*pending)
            pending = (t0, tc)
            t0 += tc
        emit_spike_out(*pending)
    nc.compile()
    return nc


def _get_nc(t_steps: int, d_imm: float):
    key = (t_steps, np.float32(d_imm).tobytes())
    if key not in _BUILD_CACHE:
        _BUILD_CACHE[key] = _build_nc(t_steps, d_imm)
    return _BUILD_CACHE[key]


def _shard_x(x: np.ndarray) -> list[np.ndarray]:
    b, t, d = x.shape
    # [b, t, core, chunk, 256] -> [core, b, chunk, t, 256] -> [core, 128, t*256]
    xr = x.reshape(b, t, N_CORES, 4, F).transpose(2, 0, 3, 1, 4)
    xr = np.ascontiguousarray(xr).reshape(N_CORES, P, t * F)
    return [xr[c] for c in range(N_CORES)]


def _unshard_spikes(s8: np.ndarray, t: int) -> np.ndarray:
    # s8 already holds spikes as u8 0/1; [core, 128, t*256] -> [b, t, D]
    sr = s8.astype(np.float32).reshape(N_CORES, B, 4, t, F)
    sr = sr.transpose(1, 3, 0, 2, 4)
    return np.ascontiguousarray(sr).reshape(B, t, N_CORES * 4 * F)


def _sigmoid_f32(decay: np.ndarray) -> np.float32:
    import jax
    import jax.numpy as jnp
    d = np.asarray(jax.nn.sigmoid(jnp.asarray(decay, jnp.float32)))
    return np.float32(d.reshape(-1)[0])


def kernel(x: np.ndarray, decay: np.ndarray) -> np.ndarray:
    from concourse.bass_utils import run_bass_kernel_spmd

    x = np.asarray(x, dtype=np.float32)
    b, t, d = x.shape
    d_f32 = _sigmoid_f32(np.asarray(decay))

    nc = _get_nc(t, float(d_f32))
    shards = _shard_x(x)
    in_maps = [{"x": np.ascontiguousarray(s)} for s in shards]
    res = run_bass_kernel_spmd(nc, in_maps, core_ids=list(range(N_CORES)))
    s8 = np.stack([np.asarray(res.results[c]["s"]) for c in range(N_CORES)],
                  axis=0)
    return _unshard_spikes(s8, t)
